# revision 9
# baseline (speedup 1.0000x reference)
"""Trainium2 Bass kernel for nn_Net_18906446037087 (snntorch Leaky SNN layer).

Reference semantics (per batch element, 255 steps, f32):
    cur = x @ W.T                         # [B, 1]
    m_0 = 0
    m_{t+1} = (0.95*m_t + cur) * (m_t <= 1)
    spk_{t+1} = (m_{t+1} > 1)
Outputs: (spk_rec, mem_rec), each [255, B, 1] f32.

Sharding: pure data parallel over batch across 8 cores (B=65536 -> 8192/core).

Key algorithmic move (v3): after a spike the membrane resets to exactly 0.0
and cur is constant, so every trajectory is EXACTLY periodic with period
p = k* + 1, where k* is the first step whose (bit-exact, iterated-f32)
value exceeds 1. The 255-step time recurrence therefore needs no
cross-instruction dependency chain at all:

 1. k* is recovered exactly from host-precomputed f32 thresholds T_k
    (largest cur with F_k(cur) <= 1, found by bisection over f32 bits):
    cmp_k = [T_k >= cur] is a 0/1 staircase, one STT per column.
 2. The per-step "no-reset" mask row z_t = [t mod p != 0] is gathered from
    a 256x256 table by telescoping (summation by parts):
    z = sum_k cmp_k * D_k with D_k = Z[p=k+2]-Z[p=k+1], D_256 = Z[p=2] --
    two PE matmuls against the ternary D table. All values are small
    integers, so float32r (1 cycle/row) is exact here.
 3. One DVE tensor_tensor_scan per column runs the actual recurrence
    state' = (beta*z)*state + (cur*z) along the free (time) axis inside a
    single instruction -- verified bit-exact vs the iterated reference,
    including exact-threshold and threshold+-1ulp cur values.

The matvec feeding cur runs in float32r end-to-end (PE transposes at
1.5 cyc/row, W-stationary matmuls at 1 cyc/row vs 4 for fp32); measured
fp32r HW error on cur is ~1.5e-4, giving ~1.7e-3 final rel-err vs the
2e-2 gate (spike-phase flips near period boundaries dominate, priced by a
direct perturbation experiment). cur reaches the partition-major layout
via 4 tiny PE transposes of the PSUM acc line (no DRAM bounce).

x is loaded in 8 big DMAs ([128, 8*784], 25 KB contiguous per partition);
every output DMA is one [128, 255] column slab (1020 B descriptors). Host
derives spk_rec (exact) and un-shuffles the output layout.
"""
import sys
if "/opt/trn_rl_repo" not in sys.path:
    sys.path.insert(0, "/opt/trn_rl_repo")

import numpy as np
from contextlib import ExitStack

import concourse.bass as bass
import concourse.bacc as bacc
import concourse.mybir as mybir
import concourse.tile as tile
from concourse.bass_utils import run_bass_kernel_spmd

F32 = mybir.dt.float32
F32R = mybir.dt.float32r
ALU = mybir.AluOpType

N_CORES = 8
B_FULL = 65536
B_CORE = B_FULL // N_CORES          # 8192
D = 784
NUM_STEPS = 255
NK = 256                             # threshold entries (255 taus + 1 big)
BETA = 0.95
THRESHOLD = 1.0

GROUP = 512                          # batch rows per matmul group (4 cols)
NGROUP = B_CORE // GROUP             # 16
CHUNKS = [(0, 128), (128, 128), (256, 128), (384, 128), (512, 128), (640, 128), (768, 16)]
COLS = B_CORE // 128                 # 64 membrane-tile columns

XROWS = 8                            # j-rows per x-load DMA
XG_BUFS = 5


def _build():
    nc = bacc.Bacc("TRN2", target_bir_lowering=False, debug=False,
                   num_devices=N_CORES)
    x_d = nc.dram_tensor("x", [B_CORE, D], F32R, kind="ExternalInput")
    w_d = nc.dram_tensor("w", [128, 7], F32R, kind="ExternalInput")
    id_d = nc.dram_tensor("ident", [128, 128], F32R, kind="ExternalInput")
    tau_d = nc.dram_tensor("tau", [128, NK], F32, kind="ExternalInput")
    ibeta_d = nc.dram_tensor("ibeta", [128, NUM_STEPS], F32, kind="ExternalInput")
    d0_d = nc.dram_tensor("d0", [128, NK], F32R, kind="ExternalInput")
    d1_d = nc.dram_tensor("d1", [128, NK], F32R, kind="ExternalInput")
    mem_d = nc.dram_tensor("mem", [COLS, 128, NUM_STEPS], F32,
                           kind="ExternalOutput")

    # x rows grouped by scan column j: element e = p*64 + j sits at
    # membrane tile [p, j]; partition p's rows j..j+XROWS are contiguous.
    x_rows = x_d[:].rearrange("(p j) f -> p j f", j=COLS)

    with tile.TileContext(nc) as tc, ExitStack() as ctx:
        xpool = ctx.enter_context(tc.tile_pool(name="xpool", bufs=XG_BUFS))
        xtpool = ctx.enter_context(tc.tile_pool(name="xtpool", bufs=3))
        linepool = ctx.enter_context(tc.tile_pool(name="linepool", bufs=2))
        cmppool = ctx.enter_context(tc.tile_pool(name="cmppool", bufs=2))
        abpool = ctx.enter_context(tc.tile_pool(name="abpool", bufs=3))
        outpool = ctx.enter_context(tc.tile_pool(name="outpool", bufs=3))
        const = ctx.enter_context(tc.tile_pool(name="const", bufs=1))
        psum = ctx.enter_context(tc.tile_pool(name="psum", bufs=2, space="PSUM"))
        psacc = ctx.enter_context(tc.tile_pool(name="psacc", bufs=1, space="PSUM"))
        pscur = ctx.enter_context(tc.tile_pool(name="pscur", bufs=1, space="PSUM"))
        pscmp = ctx.enter_context(tc.tile_pool(name="pscmp", bufs=2, space="PSUM"))
        psz = ctx.enter_context(tc.tile_pool(name="psz", bufs=2, space="PSUM"))

        w_t = const.tile([128, 7], F32R)
        id_t = const.tile([128, 128], F32R)
        id1_t = const.tile([1, 1], F32)
        tau_t = const.tile([128, NK], F32)
        ibeta_t = const.tile([128, NUM_STEPS], F32)
        d0_t = const.tile([128, NK], F32R)
        d1_t = const.tile([128, NK], F32R)
        ones_t = const.tile([128, NK], F32)
        nc.sync.dma_start(w_t[:], w_d[:])
        nc.sync.dma_start(id_t[:], id_d[:])
        nc.sync.dma_start(tau_t[:], tau_d[:])
        nc.sync.dma_start(ibeta_t[:], ibeta_d[:])
        nc.sync.dma_start(d0_t[:], d0_d[:])
        nc.sync.dma_start(d1_t[:], d1_d[:])
        nc.vector.memset(id1_t[:], 1.0)
        nc.vector.memset(ones_t[:], 1.0)

        cur_t = const.tile([128, COLS], F32, name="cur")

        xtiles = {}

        def load_xtile(xi):
            xt_ = xpool.tile([128, XROWS * D], F32R, tag="xg")
            nc.sync.dma_start(
                xt_[:].rearrange("p (j f) -> p j f", j=XROWS),
                x_rows[:, xi * XROWS:(xi + 1) * XROWS],
            )
            xtiles[xi] = xt_

        def matvec_group(g):
            """cur for batch columns [4g, 4g+4)."""
            xi, r0 = (4 * g) // XROWS, (4 * g) % XROWS
            xg = xtiles[xi]
            acc = psacc.tile([1, GROUP], F32, tag="acc")
            for ci, (c0, cl) in enumerate(CHUNKS):
                xt_ps = psum.tile([128, GROUP], F32R, tag="xt")
                for t in range(4):
                    nc.tensor.transpose(
                        xt_ps[:cl, t * 128:(t + 1) * 128],
                        xg[:, (r0 + t) * D + c0:(r0 + t) * D + c0 + cl],
                        id_t[:],
                    )
                xt_sb = xtpool.tile([128, GROUP], F32R, tag="xtsb")
                if ci == 3 or (ci == 5 and g % 2 == 1):
                    nc.vector.tensor_copy(xt_sb[:cl, :], xt_ps[:cl, :])
                else:
                    nc.scalar.copy(xt_sb[:cl, :], xt_ps[:cl, :])
                nc.tensor.matmul(
                    acc[:, :],
                    w_t[:cl, ci:ci + 1],
                    xt_sb[:cl, :],
                    start=(ci == 0),
                    stop=(ci == len(CHUNKS) - 1),
                )
            # acc[0, t*128 + p] = cur[e = p*64 + 4g + t]; transpose each
            # 128-slice onto partitions to land cur in scan layout.
            line = linepool.tile([1, GROUP], F32, tag="line")
            nc.scalar.copy(line[:, :], acc[:, :])
            cur_ps = pscur.tile([128, 4], F32, tag="curps")
            for t in range(4):
                nc.tensor.transpose(
                    cur_ps[:, t:t + 1],
                    line[:, t * 128:(t + 1) * 128],
                    id1_t[:],
                )
            nc.scalar.copy(cur_t[:, 4 * g:4 * g + 4], cur_ps[:, :])

        def column(j):
            """Expand column j's full 255-step trajectory (no time chain)."""
            cur_j = cur_t[:, j:j + 1]
            cmp_t = cmppool.tile([128, NK], F32R, tag="cmp")
            nc.vector.scalar_tensor_tensor(
                cmp_t[:], tau_t[:], cur_j, ones_t[:], ALU.is_ge, ALU.bypass)
            cmpT_ps = pscmp.tile([128, NK], F32R, tag="cmpT")
            nc.tensor.transpose(cmpT_ps[:, 0:128], cmp_t[:, 0:128], id_t[:])
            nc.tensor.transpose(cmpT_ps[:, 128:256], cmp_t[:, 128:256], id_t[:])
            cmpT_sb = cmppool.tile([128, NK], F32R, tag="cmpTsb")
            nc.scalar.copy(cmpT_sb[:], cmpT_ps[:])
            z_ps = psz.tile([128, NK], F32, tag="z")
            nc.tensor.matmul(z_ps[:], cmpT_sb[:, 0:128], d0_t[:],
                             start=True, stop=False)
            nc.tensor.matmul(z_ps[:], cmpT_sb[:, 128:256], d1_t[:],
                             start=False, stop=True)
            # scaled state s_t = mem_t * beta^-t: s' = z*s + z*cur*beta^-(t+1);
            # host multiplies by beta^t afterwards. Kills the a = beta*z pass.
            b_t = abpool.tile([128, NUM_STEPS], F32, tag="b")
            nc.vector.scalar_tensor_tensor(
                b_t[:], z_ps[:, 0:NUM_STEPS], cur_j, ibeta_t[:],
                ALU.mult, ALU.mult)
            m_t = outpool.tile([128, NUM_STEPS], F32, tag="m")
            nc.vector.tensor_tensor_scan(m_t[:], z_ps[:, 0:NUM_STEPS], b_t[:],
                                         0.0, ALU.mult, ALU.add)
            nc.gpsimd.dma_start(mem_d[j], m_t[:])

        for xi in range(B_CORE // (128 * XROWS)):
            load_xtile(xi)
        for g in range(NGROUP):
            matvec_group(g)
            for j in range(4 * g, 4 * g + 4):
                column(j)

    nc.compile()
    return nc


_NC_CACHE = None
_TABLE_CACHE = None


def _get_nc():
    global _NC_CACHE
    if _NC_CACHE is None:
        _NC_CACHE = _build()
    return _NC_CACHE


def _tables():
    """Exact f32 thresholds T_k of the iterated recurrence + the telescoped
    no-reset-mask difference table."""
    global _TABLE_CACHE
    if _TABLE_CACHE is not None:
        return _TABLE_CACHE
    T = NUM_STEPS
    beta = np.float32(BETA)
    # T_k = largest f32 cur with F_k(cur) <= 1; F_1 = cur,
    # F_{j+1} = f32(f32(beta*F_j) + cur). Bisect on f32 bit patterns,
    # vectorized over k (candidate k sits at vector slot k-1 and needs
    # F_k, i.e. k-1 update steps).
    lo = np.full(T, np.float32(0.04), np.float32).view(np.uint32).copy()
    hi = np.full(T, np.float32(1.5), np.float32).view(np.uint32).copy()
    for _ in range(40):
        mid = ((lo.astype(np.uint64) + hi.astype(np.uint64)) // 2).astype(np.uint32)
        cur = mid.view(np.float32)
        traj = cur.copy()
        fk = np.empty(T, np.float32)
        fk[0] = traj[0]
        for j in range(1, T):
            traj = ((beta * traj).astype(np.float32) + cur).astype(np.float32)
            fk[j] = traj[j]
        ok = fk <= np.float32(1.0)
        lo = np.where(ok, mid, lo)
        hi = np.where(ok, hi, mid)
        if np.all(hi - lo <= 1):
            break
    taus = lo.view(np.float32).copy()
    tau_row = np.concatenate([taus, np.array([3e38], np.float32)])
    # Z[p-2, t-1] = [t mod p != 0] for p = 2..257, t = 1..256
    pvals = np.arange(2, 258)
    tvals = np.arange(1, 257)
    Z = ((tvals[None, :] % pvals[:, None]) != 0).astype(np.float32)
    Dm = np.zeros((256, 256), np.float32)
    Dm[0:255] = Z[1:256] - Z[0:255]
    Dm[255] = Z[0]
    r = (np.float64(BETA) ** -np.arange(1, NUM_STEPS + 1)).astype(np.float32)
    g = (1.0 / r.astype(np.float64))
    _TABLE_CACHE = (np.tile(tau_row, (128, 1)),
                    np.ascontiguousarray(Dm[:128]),
                    np.ascontiguousarray(Dm[128:]),
                    np.tile(r, (128, 1)), g)
    return _TABLE_CACHE


def _prep_inputs(x, W):
    x = np.ascontiguousarray(np.asarray(x, dtype=np.float32))
    W = np.asarray(W, dtype=np.float32).reshape(-1)
    assert x.shape == (B_FULL, D) and W.shape == (D,)
    wpad = np.zeros(896, np.float32)
    wpad[:D] = W
    wcol = np.ascontiguousarray(wpad.reshape(7, 128).T)
    ident = np.eye(128, dtype=np.float32)
    tau, d0, d1, ibeta, _ = _tables()
    in_maps = [
        {"x": x[d * B_CORE:(d + 1) * B_CORE], "w": wcol, "ident": ident,
         "tau": tau, "d0": d0, "d1": d1, "ibeta": ibeta}
        for d in range(N_CORES)
    ]
    return in_maps


def kernel(x, W, _trace=False, _trace_kwargs=None):
    nc = _get_nc()
    in_maps = _prep_inputs(x, W)
    res = run_bass_kernel_spmd(nc, in_maps, list(range(N_CORES)),
                               trace=_trace, **(_trace_kwargs or {}))
    g = _tables()[4]  # beta^t rescale of the scaled-state output
    mem = np.empty((NUM_STEPS, B_FULL), np.float32)
    for d in range(N_CORES):
        # [COLS, 128, T] -> mem[t, p*64 + j], rescaled by beta^t
        arr = res.results[d]["mem"].astype(np.float64) * g[None, None, :]
        mem[:, d * B_CORE:(d + 1) * B_CORE] = \
            arr.transpose(2, 1, 0).reshape(NUM_STEPS, B_CORE).astype(np.float32)
    mem_rec = mem.reshape(NUM_STEPS, B_FULL, 1)
    spk_rec = (mem_rec > np.float32(THRESHOLD)).astype(np.float32)
    if _trace:
        return (spk_rec, mem_rec), res
    return spk_rec, mem_rec


# revision 10
# speedup vs baseline: 1.1009x; 1.1009x over previous
"""Trainium2 Bass kernel for nn_Net_18906446037087 (snntorch Leaky SNN layer).

Reference semantics (per batch element, 255 steps, f32):
    cur = x @ W.T                         # [B, 1]
    m_0 = 0
    m_{t+1} = (0.95*m_t + cur) * (m_t <= 1)
    spk_{t+1} = (m_{t+1} > 1)
Outputs: (spk_rec, mem_rec), each [255, B, 1] f32.

Sharding: pure data parallel over batch across 8 cores (B=65536 -> 8192/core).

Key algorithmic move (v3): after a spike the membrane resets to exactly 0.0
and cur is constant, so every trajectory is EXACTLY periodic with period
p = k* + 1, where k* is the first step whose (bit-exact, iterated-f32)
value exceeds 1. The 255-step time recurrence therefore needs no
cross-instruction dependency chain at all:

 1. k* is recovered exactly from host-precomputed f32 thresholds T_k
    (largest cur with F_k(cur) <= 1, found by bisection over f32 bits):
    cmp_k = [T_k >= cur] is a 0/1 staircase, one STT per column.
 2. The per-step "no-reset" mask row z_t = [t mod p != 0] is gathered from
    a 256x256 table by telescoping (summation by parts):
    z = sum_k cmp_k * D_k with D_k = Z[p=k+2]-Z[p=k+1], D_256 = Z[p=2] --
    two PE matmuls against the ternary D table. All values are small
    integers, so float32r (1 cycle/row) is exact here.
 3. One DVE tensor_tensor_scan per column runs the actual recurrence
    state' = (beta*z)*state + (cur*z) along the free (time) axis inside a
    single instruction -- verified bit-exact vs the iterated reference,
    including exact-threshold and threshold+-1ulp cur values.

The matvec feeding cur runs in float32r end-to-end (PE transposes at
1.5 cyc/row, W-stationary matmuls at 1 cyc/row vs 4 for fp32); measured
fp32r HW error on cur is ~1.5e-4, giving ~1.7e-3 final rel-err vs the
2e-2 gate (spike-phase flips near period boundaries dominate, priced by a
direct perturbation experiment). cur reaches the partition-major layout
via 4 tiny PE transposes of the PSUM acc line (no DRAM bounce).

x is loaded in 8 big DMAs ([128, 8*784], 25 KB contiguous per partition);
every output DMA is one [128, 255] column slab (1020 B descriptors). Host
derives spk_rec (exact) and un-shuffles the output layout.
"""
import sys
if "/opt/trn_rl_repo" not in sys.path:
    sys.path.insert(0, "/opt/trn_rl_repo")

import numpy as np
from contextlib import ExitStack

import concourse.bass as bass
import concourse.bacc as bacc
import concourse.mybir as mybir
import concourse.tile as tile
from concourse.bass_utils import run_bass_kernel_spmd

F32 = mybir.dt.float32
F32R = mybir.dt.float32r
ALU = mybir.AluOpType

N_CORES = 8
B_FULL = 65536
B_CORE = B_FULL // N_CORES          # 8192
D = 784
NUM_STEPS = 255
NK = 256                             # threshold entries (255 taus + 1 big)
BETA = 0.95
THRESHOLD = 1.0

GROUP = 512                          # batch rows per matmul group (4 cols)
NGROUP = B_CORE // GROUP             # 16
CHUNKS = [(0, 128), (128, 128), (256, 128), (384, 128), (512, 128), (640, 128), (768, 16)]
COLS = B_CORE // 128                 # 64 membrane-tile columns

XROWS = 8                            # j-rows per x-load DMA
XG_BUFS = 5


def _build():
    nc = bacc.Bacc("TRN2", target_bir_lowering=False, debug=False,
                   num_devices=N_CORES)
    x_d = nc.dram_tensor("x", [B_CORE, D], F32R, kind="ExternalInput")
    w_d = nc.dram_tensor("w", [128, 7], F32R, kind="ExternalInput")
    id_d = nc.dram_tensor("ident", [128, 128], F32R, kind="ExternalInput")
    tau_d = nc.dram_tensor("tau", [128, NK], F32, kind="ExternalInput")
    ibeta_d = nc.dram_tensor("ibeta", [128, NUM_STEPS], F32, kind="ExternalInput")
    d0_d = nc.dram_tensor("d0", [128, NK], F32R, kind="ExternalInput")
    d1_d = nc.dram_tensor("d1", [128, NK], F32R, kind="ExternalInput")
    mem_d = nc.dram_tensor("mem", [COLS, 128, NUM_STEPS], F32,
                           kind="ExternalOutput")

    # x rows grouped by scan column j: element e = p*64 + j sits at
    # membrane tile [p, j]; partition p's rows j..j+XROWS are contiguous.
    x_rows = x_d[:].rearrange("(p j) f -> p j f", j=COLS)

    with tile.TileContext(nc) as tc, ExitStack() as ctx:
        xpool = ctx.enter_context(tc.tile_pool(name="xpool", bufs=XG_BUFS))
        xtpool = ctx.enter_context(tc.tile_pool(name="xtpool", bufs=3))
        linepool = ctx.enter_context(tc.tile_pool(name="linepool", bufs=2))
        cmppool = ctx.enter_context(tc.tile_pool(name="cmppool", bufs=2))
        abpool = ctx.enter_context(tc.tile_pool(name="abpool", bufs=3))
        outpool = ctx.enter_context(tc.tile_pool(name="outpool", bufs=3))
        const = ctx.enter_context(tc.tile_pool(name="const", bufs=1))
        psum = ctx.enter_context(tc.tile_pool(name="psum", bufs=2, space="PSUM"))
        psacc = ctx.enter_context(tc.tile_pool(name="psacc", bufs=1, space="PSUM"))
        pscur = ctx.enter_context(tc.tile_pool(name="pscur", bufs=1, space="PSUM"))
        pscmp = ctx.enter_context(tc.tile_pool(name="pscmp", bufs=2, space="PSUM"))
        psz = ctx.enter_context(tc.tile_pool(name="psz", bufs=2, space="PSUM"))

        w_t = const.tile([128, 7], F32R)
        id_t = const.tile([128, 128], F32R)
        id1_t = const.tile([1, 1], F32)
        tau_t = const.tile([128, NK], F32)
        ibeta_t = const.tile([128, NUM_STEPS], F32)
        d0_t = const.tile([128, NK], F32R)
        d1_t = const.tile([128, NK], F32R)
        ones_t = const.tile([128, NK], F32)
        nc.sync.dma_start(w_t[:], w_d[:])
        nc.sync.dma_start(id_t[:], id_d[:])
        nc.sync.dma_start(tau_t[:], tau_d[:])
        nc.sync.dma_start(ibeta_t[:], ibeta_d[:])
        nc.sync.dma_start(d0_t[:], d0_d[:])
        nc.sync.dma_start(d1_t[:], d1_d[:])
        nc.vector.memset(id1_t[:], 1.0)
        nc.vector.memset(ones_t[:], 1.0)

        cur_t = const.tile([128, COLS], F32, name="cur")

        xtiles = {}

        def load_xtile(xi):
            xt_ = xpool.tile([128, XROWS * D], F32R, tag="xg")
            nc.sync.dma_start(
                xt_[:].rearrange("p (j f) -> p j f", j=XROWS),
                x_rows[:, xi * XROWS:(xi + 1) * XROWS],
            )
            xtiles[xi] = xt_

        def matvec_group(g):
            """cur for batch columns [4g, 4g+4)."""
            xi, r0 = (4 * g) // XROWS, (4 * g) % XROWS
            xg = xtiles[xi]
            acc = psacc.tile([1, GROUP], F32, tag="acc")
            for ci, (c0, cl) in enumerate(CHUNKS):
                xt_ps = psum.tile([128, GROUP], F32R, tag="xt")
                for t in range(4):
                    nc.tensor.transpose(
                        xt_ps[:cl, t * 128:(t + 1) * 128],
                        xg[:, (r0 + t) * D + c0:(r0 + t) * D + c0 + cl],
                        id_t[:],
                    )
                xt_sb = xtpool.tile([128, GROUP], F32R, tag="xtsb")
                if ci == 3 or (ci == 5 and g % 2 == 1):
                    nc.vector.tensor_copy(xt_sb[:cl, :], xt_ps[:cl, :])
                else:
                    nc.scalar.copy(xt_sb[:cl, :], xt_ps[:cl, :])
                nc.tensor.matmul(
                    acc[:, :],
                    w_t[:cl, ci:ci + 1],
                    xt_sb[:cl, :],
                    start=(ci == 0),
                    stop=(ci == len(CHUNKS) - 1),
                )
            # acc[0, t*128 + p] = cur[e = p*64 + 4g + t]; transpose each
            # 128-slice onto partitions to land cur in scan layout.
            line = linepool.tile([1, GROUP], F32, tag="line")
            nc.scalar.copy(line[:, :], acc[:, :])
            cur_ps = pscur.tile([128, 4], F32, tag="curps")
            for t in range(4):
                nc.tensor.transpose(
                    cur_ps[:, t:t + 1],
                    line[:, t * 128:(t + 1) * 128],
                    id1_t[:],
                )
            nc.scalar.copy(cur_t[:, 4 * g:4 * g + 4], cur_ps[:, :])

        def column(j):
            """Expand column j's full 255-step trajectory (no time chain)."""
            cur_j = cur_t[:, j:j + 1]
            cmp_t = cmppool.tile([128, NK], F32R, tag="cmp")
            nc.vector.scalar_tensor_tensor(
                cmp_t[:], tau_t[:], cur_j, ones_t[:], ALU.is_ge, ALU.bypass)
            cmpT_ps = pscmp.tile([128, NK], F32R, tag="cmpT")
            nc.tensor.transpose(cmpT_ps[:, 0:128], cmp_t[:, 0:128], id_t[:])
            nc.tensor.transpose(cmpT_ps[:, 128:256], cmp_t[:, 128:256], id_t[:])
            cmpT_sb = cmppool.tile([128, NK], F32R, tag="cmpTsb")
            nc.scalar.copy(cmpT_sb[:], cmpT_ps[:])
            z_ps = psz.tile([128, NK], F32, tag="z")
            nc.tensor.matmul(z_ps[:], cmpT_sb[:, 0:128], d0_t[:],
                             start=True, stop=False)
            nc.tensor.matmul(z_ps[:], cmpT_sb[:, 128:256], d1_t[:],
                             start=False, stop=True)
            # scaled state s_t = mem_t * beta^-t: s' = z*s + z*cur*beta^-(t+1);
            # host multiplies by beta^t afterwards. Kills the a = beta*z pass.
            b_t = abpool.tile([128, NUM_STEPS], F32, tag="b")
            nc.vector.scalar_tensor_tensor(
                b_t[:], z_ps[:, 0:NUM_STEPS], cur_j, ibeta_t[:],
                ALU.mult, ALU.mult)
            m_t = outpool.tile([128, NUM_STEPS], F32, tag="m")
            nc.vector.tensor_tensor_scan(m_t[:], z_ps[:, 0:NUM_STEPS], b_t[:],
                                         0.0, ALU.mult, ALU.add)
            nc.sync.dma_start(mem_d[j], m_t[:])

        for xi in range(B_CORE // (128 * XROWS)):
            load_xtile(xi)
        for g in range(NGROUP):
            matvec_group(g)
            for j in range(4 * g, 4 * g + 4):
                column(j)

    nc.compile()
    return nc


_NC_CACHE = None
_TABLE_CACHE = None


def _get_nc():
    global _NC_CACHE
    if _NC_CACHE is None:
        _NC_CACHE = _build()
    return _NC_CACHE


def _tables():
    """Exact f32 thresholds T_k of the iterated recurrence + the telescoped
    no-reset-mask difference table."""
    global _TABLE_CACHE
    if _TABLE_CACHE is not None:
        return _TABLE_CACHE
    T = NUM_STEPS
    beta = np.float32(BETA)
    # T_k = largest f32 cur with F_k(cur) <= 1; F_1 = cur,
    # F_{j+1} = f32(f32(beta*F_j) + cur). Bisect on f32 bit patterns,
    # vectorized over k (candidate k sits at vector slot k-1 and needs
    # F_k, i.e. k-1 update steps).
    lo = np.full(T, np.float32(0.04), np.float32).view(np.uint32).copy()
    hi = np.full(T, np.float32(1.5), np.float32).view(np.uint32).copy()
    for _ in range(40):
        mid = ((lo.astype(np.uint64) + hi.astype(np.uint64)) // 2).astype(np.uint32)
        cur = mid.view(np.float32)
        traj = cur.copy()
        fk = np.empty(T, np.float32)
        fk[0] = traj[0]
        for j in range(1, T):
            traj = ((beta * traj).astype(np.float32) + cur).astype(np.float32)
            fk[j] = traj[j]
        ok = fk <= np.float32(1.0)
        lo = np.where(ok, mid, lo)
        hi = np.where(ok, hi, mid)
        if np.all(hi - lo <= 1):
            break
    taus = lo.view(np.float32).copy()
    tau_row = np.concatenate([taus, np.array([3e38], np.float32)])
    # Z[p-2, t-1] = [t mod p != 0] for p = 2..257, t = 1..256
    pvals = np.arange(2, 258)
    tvals = np.arange(1, 257)
    Z = ((tvals[None, :] % pvals[:, None]) != 0).astype(np.float32)
    Dm = np.zeros((256, 256), np.float32)
    Dm[0:255] = Z[1:256] - Z[0:255]
    Dm[255] = Z[0]
    r = (np.float64(BETA) ** -np.arange(1, NUM_STEPS + 1)).astype(np.float32)
    g = (1.0 / r.astype(np.float64))
    _TABLE_CACHE = (np.tile(tau_row, (128, 1)),
                    np.ascontiguousarray(Dm[:128]),
                    np.ascontiguousarray(Dm[128:]),
                    np.tile(r, (128, 1)), g)
    return _TABLE_CACHE


def _prep_inputs(x, W):
    x = np.ascontiguousarray(np.asarray(x, dtype=np.float32))
    W = np.asarray(W, dtype=np.float32).reshape(-1)
    assert x.shape == (B_FULL, D) and W.shape == (D,)
    wpad = np.zeros(896, np.float32)
    wpad[:D] = W
    wcol = np.ascontiguousarray(wpad.reshape(7, 128).T)
    ident = np.eye(128, dtype=np.float32)
    tau, d0, d1, ibeta, _ = _tables()
    in_maps = [
        {"x": x[d * B_CORE:(d + 1) * B_CORE], "w": wcol, "ident": ident,
         "tau": tau, "d0": d0, "d1": d1, "ibeta": ibeta}
        for d in range(N_CORES)
    ]
    return in_maps


def kernel(x, W, _trace=False, _trace_kwargs=None):
    nc = _get_nc()
    in_maps = _prep_inputs(x, W)
    res = run_bass_kernel_spmd(nc, in_maps, list(range(N_CORES)),
                               trace=_trace, **(_trace_kwargs or {}))
    g = _tables()[4]  # beta^t rescale of the scaled-state output
    mem = np.empty((NUM_STEPS, B_FULL), np.float32)
    for d in range(N_CORES):
        # [COLS, 128, T] -> mem[t, p*64 + j], rescaled by beta^t
        arr = res.results[d]["mem"].astype(np.float64) * g[None, None, :]
        mem[:, d * B_CORE:(d + 1) * B_CORE] = \
            arr.transpose(2, 1, 0).reshape(NUM_STEPS, B_CORE).astype(np.float32)
    mem_rec = mem.reshape(NUM_STEPS, B_FULL, 1)
    spk_rec = (mem_rec > np.float32(THRESHOLD)).astype(np.float32)
    if _trace:
        return (spk_rec, mem_rec), res
    return spk_rec, mem_rec
